# revision 1
# baseline (speedup 1.0000x reference)
"""Trainium2 Bass kernel for nn_Attention_65317862638379.

Dense transformer block-attention with per-token geometric (rotation+translation)
transform. B=16, N=2048, DIM=1024, H=16, DH=64; attention over N/4=512 block
tokens of dim 256.

Sharding: data-parallel over batch, 2 batches per core, 8 cores, no collectives.
All matmuls run in bf16 on the PE (fp32 PSUM accumulation).

Layouts (per batch, per core):
  - Q/K: transposed projection -> qkT [j=(head,dh) rows, t] ; fwd rotation done
    with stream_shuffle (partition pair swap) + cosT/sinT coefficient tiles.
  - V: a-split natural projection -> Vb [J, (h,a,dh)] block layout; fwd rotation
    on the free axis + translation.
  - Attention simT[J, I] per head; softmax without max-subtraction (scores are
    O(5)); denominators via ones-matmul (replicated across partitions).
  - PV -> A [(a,dh), I]; normalize, inverse translate/rotate, write attn_outT
    [j, t]; final projection back to natural [t, e].
"""

import numpy as np
import ml_dtypes

import concourse.bass as bass
import concourse.mybir as mybir
import concourse.tile as tile
from concourse.bass_utils import run_bass_kernel_spmd

BF16 = ml_dtypes.bfloat16

B, N, DIM, H, DH = 16, 2048, 1024, 16, 64
D_FLAT, D_ROT, NPAIR = 32, 32, 16
BLK = 4
NB = N // BLK          # 512 block tokens
DB = DH * BLK          # 256 block dim
NCORES = 8
B2 = B // NCORES       # batches per core
SCALE = float((DH * BLK) ** -0.5)  # 1/16, TAU=1.0

FP32 = mybir.dt.float32
BFD = mybir.dt.bfloat16

MULT = mybir.AluOpType.mult
ADD = mybir.AluOpType.add
SUB = mybir.AluOpType.subtract

_CACHE = {}


def _split_multi_waits(nc):
    """walrus codegen only supports one sync-wait per instruction; hoist
    extra waits onto preceding same-engine NoOps."""
    cnt = 0
    for f in nc.m.functions:
        for blk in f.blocks:
            insts = blk.instructions
            out = []
            for inst in insts:
                si = inst.sync_info
                if si is not None and si.on_wait and len(si.on_wait) > 1:
                    waits = list(si.on_wait)
                    for w in waits[:-1]:
                        cnt += 1
                        nop = mybir.InstNoOp(name=f"WSPLIT-{cnt}", ins=[], outs=[])
                        nop.engine = inst.engine
                        nop.sync_info = mybir.SyncInfo(on_wait=[w], on_update=[])
                        out.append(nop)
                    inst.sync_info = mybir.SyncInfo(
                        on_wait=[waits[-1]], on_update=list(si.on_update))
                out.append(inst)
            blk.instructions = out
    return cnt


def _build_nc():
    """Build the Bass graph (SPMD; same NEFF on all 8 cores)."""
    nc = bass.Bass(target_bir_lowering=False)

    # ---------------- DRAM parameters (per-core shapes) ----------------
    xT_d = nc.dram_tensor("xT", [B2, DIM, N], BFD, kind="ExternalInput")
    wqkvT_d = nc.dram_tensor("wqkvT", [DIM, 3 * H * DH], BFD, kind="ExternalInput")
    woutT_d = nc.dram_tensor("woutT", [DIM, DIM], BFD, kind="ExternalInput")
    boutB_d = nc.dram_tensor("boutB", [128, DIM], FP32, kind="ExternalInput")
    cosT_d = nc.dram_tensor("cosT", [B2, 128, N], BFD, kind="ExternalInput")
    sinT_d = nc.dram_tensor("sinT", [B2, 128, N], BFD, kind="ExternalInput")
    cstN_d = nc.dram_tensor("cstN", [B2, BLK, NB, 512], BFD, kind="ExternalInput")
    cosE_d = nc.dram_tensor("cosE", [B2, 2, 128, NB], BFD, kind="ExternalInput")
    sinE_d = nc.dram_tensor("sinE", [B2, 2, 128, NB], BFD, kind="ExternalInput")
    transB_d = nc.dram_tensor("transB", [B2, 2, 128, NB], BFD, kind="ExternalInput")

    out_d = nc.dram_tensor("out", [B2, N, DIM], BFD, kind="ExternalOutput")

    swap_mask = []
    for i in range(16):
        swap_mask += [2 * i + 1, 2 * i]

    from contextlib import ExitStack
    with ExitStack() as ctx:
        tc = ctx.enter_context(tile.TileContext(nc))
        ep = ctx.enter_context
        consts = ep(tc.tile_pool(name="consts", bufs=1))
        xT_pool = ep(tc.tile_pool(name="xT", bufs=1))
        wv_pool = ep(tc.tile_pool(name="wv", bufs=1))
        wqk_pool = ep(tc.tile_pool(name="wqk", bufs=1))
        wout_pool = ep(tc.tile_pool(name="wout", bufs=1))
        vb_pool = ep(tc.tile_pool(name="vb", bufs=1))
        qk_pool = ep(tc.tile_pool(name="qk", bufs=2))
        ao_pool = ep(tc.tile_pool(name="ao", bufs=1))
        coef_pool = ep(tc.tile_pool(name="coefs", bufs=1))
        cn_pool = ep(tc.tile_pool(name="cn", bufs=3))
        exp_pool = ep(tc.tile_pool(name="expt", bufs=6))
        tmps_pool = ep(tc.tile_pool(name="tmps", bufs=3))
        tmpb_pool = ep(tc.tile_pool(name="tmpb", bufs=3))
        tmpa_pool = ep(tc.tile_pool(name="tmpa", bufs=2))
        oev_pool = ep(tc.tile_pool(name="oev", bufs=3))
        ps_pool = ep(tc.tile_pool(name="ps", bufs=6, space="PSUM"))
        psp_pool = ep(tc.tile_pool(name="psp", bufs=2, space="PSUM"))
        if True:
            # ---- constants ----
            ones_sb = consts.tile([128, 128], BFD)
            nc.vector.memset(ones_sb, 1.0)
            bout_sb = consts.tile([128, DIM], FP32)
            wout_sb = []

            def load_wout():
                nc.sync.dma_start(out=bout_sb, in_=boutB_d[:, :])
                for jc in range(8):
                    wt = wout_pool.tile([128, DIM], BFD, tag=f"wout{jc}", name=f"wout{jc}")
                    nc.sync.dma_start(out=wt, in_=woutT_d[jc * 128:(jc + 1) * 128, :])
                    wout_sb.append(wt)

            wv_sb = []
            for jsl in range(2):
                wvt = wv_pool.tile([128, 8, 512], BFD, tag=f"wv{jsl}", name=f"wv{jsl}")
                for dk in range(8):
                    nc.sync.dma_start(
                        out=wvt[:, dk, :],
                        in_=wqkvT_d[dk * 128:(dk + 1) * 128,
                                    2048 + jsl * 512: 2048 + (jsl + 1) * 512])
                wv_sb.append(wvt)

            for b in range(B2):
                # ---- load xT for this batch: 8 d-chunk tiles [128, 2048] ----
                xT_sb = []
                for dk in range(8):
                    t = xT_pool.tile([128, N], BFD, tag=f"xT{dk}", name=f"xT{dk}")
                    deng = [nc.sync, nc.gpsimd, nc.scalar][dk % 3]
                    deng.dma_start(
                        out=t, in_=xT_d[b, dk * 128:(dk + 1) * 128, :])
                    xT_sb.append(t)

                # ---- per-batch coefficient tiles ----
                def load_coefs():
                    cosT_sb = coef_pool.tile([128, N], BFD, tag="cosT", name="cosT")
                    sinT_sb = coef_pool.tile([128, N], BFD, tag="sinT", name="sinT")
                    nc.sync.dma_start(out=cosT_sb, in_=cosT_d[b])
                    nc.sync.dma_start(out=sinT_sb, in_=sinT_d[b])
                    cosE_sb, sinE_sb, transB_sb = [], [], []
                    for c2 in range(2):
                        ce = coef_pool.tile([128, NB], BFD, tag=f"cosE{c2}", name=f"cosE{c2}")
                        se = coef_pool.tile([128, NB], BFD, tag=f"sinE{c2}", name=f"sinE{c2}")
                        tb = coef_pool.tile([128, NB], BFD, tag=f"transB{c2}", name=f"transB{c2}")
                        nc.sync.dma_start(out=ce, in_=cosE_d[b, c2])
                        nc.sync.dma_start(out=se, in_=sinE_d[b, c2])
                        nc.sync.dma_start(out=tb, in_=transB_d[b, c2])
                        cosE_sb.append(ce)
                        sinE_sb.append(se)
                        transB_sb.append(tb)
                    return cosT_sb, sinT_sb, cosE_sb, sinE_sb, transB_sb

                # ================= V projection (a-split, natural) ==========
                # Vb store: per J-chunk tile [128, (h,a,dh)=4096]
                vb_sb = []
                for jc in range(4):
                    vt = vb_pool.tile([128, H * BLK * DH], BFD, tag=f"vb{jc}")
                    vb_sb.append(vt)

                for a in range(BLK):
                    for c in range(4):  # J-chunk
                        cst_c = cn_pool.tile([128, 512], BFD, tag="cstN")
                        nc.sync.dma_start(out=cst_c, in_=cstN_d[b, a, c * 128:(c + 1) * 128, :])
                        cn_v = cst_c[:, 0:128].rearrange("p (h i) -> p h i", h=8)
                        sn_v = cst_c[:, 128:256].rearrange("p (h i) -> p h i", h=8)
                        tn_v = cst_c[:, 256:512].rearrange("p (h i t) -> p h i t", h=8, i=16, t=2)

                        for jsl in range(2):  # v column slice (8 heads each)
                            ps = psp_pool.tile([128, 512], FP32, tag="psp")
                            for dk in range(8):
                                lhsT = xT_sb[dk].rearrange(
                                    "p (c j a) -> p c j a", c=4, j=128, a=4)[:, c, :, a]
                                nc.tensor.matmul(
                                    ps, lhsT, wv_sb[jsl][:, dk, :],
                                    start=(dk == 0), stop=(dk == 7))
                            # --- evict (ACT) then rotate + translate on GpSimd ---
                            pvr = tmps_pool.tile([128, 512], BFD, tag="pvr")
                            pv = pvr.rearrange(
                                "p (h half i t) -> p h half i t", h=8, half=2, i=16, t=2)
                            nc.scalar.copy(
                                out=pv[:, :, 1],
                                in_=ps.rearrange(
                                    "p (h half i t) -> p h half i t",
                                    h=8, half=2, i=16, t=2)[:, :, 1])
                            x0 = pv[:, :, 1, :, 0]
                            x1 = pv[:, :, 1, :, 1]
                            dst = vb_sb[c].rearrange(
                                "p (h a half i t) -> p h a half i t",
                                h=16, a=4, half=2, i=16, t=2)
                            hlo, hhi = jsl * 8, (jsl + 1) * 8
                            dflat = dst[:, hlo:hhi, a, 0]
                            de = dst[:, hlo:hhi, a, 1, :, 0]
                            do = dst[:, hlo:hhi, a, 1, :, 1]
                            nc.scalar.copy(out=dflat, in_=ps.rearrange("p (h half i t) -> p h half i t", h=8, half=2, i=16, t=2)[:, :, 0])
                            t0 = tmps_pool.tile([128, 8, 16], BFD, tag="t0")
                            t1 = tmps_pool.tile([128, 8, 16], BFD, tag="t1")
                            t4 = tmps_pool.tile([128, 8, 16], BFD, tag="t4")
                            veng = nc.vector if (a * 4 + c) % 3 else nc.gpsimd
                            veng.tensor_tensor(t0, x0, cn_v, MULT)
                            veng.tensor_tensor(t1, x1, sn_v, MULT)
                            veng.tensor_tensor(t4, t0, t1, SUB)
                            # even_rot = x0 cos - x1 sin + c*trans_even
                            veng.tensor_tensor(de, t4, tn_v[:, :, :, 0], ADD)
                            t2 = tmps_pool.tile([128, 8, 16], BFD, tag="t2")
                            t3 = tmps_pool.tile([128, 8, 16], BFD, tag="t3")
                            t5 = tmps_pool.tile([128, 8, 16], BFD, tag="t5")
                            veng.tensor_tensor(t2, x0, sn_v, MULT)
                            veng.tensor_tensor(t3, x1, cn_v, MULT)
                            veng.tensor_tensor(t5, t2, t3, ADD)
                            veng.tensor_tensor(do, t5, tn_v[:, :, :, 1], ADD)

                # ================= Q/K pairs + attention ====================
                cosT_sb, sinT_sb, cosE_sb, sinE_sb, transB_sb = load_coefs()
                if b == 0:
                    load_wout()
                ao_sb = []
                for c2 in range(8):
                    at = ao_pool.tile([128, N], BFD, tag=f"ao{c2}")
                    ao_sb.append(at)

                def emit_proj(c2):
                    qk_tiles = {}
                    for which, jc in (("q", c2), ("k", 8 + c2)):
                        wq_sb = wqk_pool.tile([128, 8, 128], BFD, tag=f"wqk_{which}")
                        for dk in range(8):
                            nc.sync.dma_start(
                                out=wq_sb[:, dk, :],
                                in_=wqkvT_d[dk * 128:(dk + 1) * 128,
                                            jc * 128:(jc + 1) * 128])
                        qt = qk_pool.tile([128, N], BFD, tag=which)
                        qk_tiles[which] = qt
                        for ts in range(4):
                            ps = ps_pool.tile([128, 512], FP32, tag="ps")
                            for dk in range(8):
                                nc.tensor.matmul(
                                    ps, wq_sb[:, dk, :],
                                    xT_sb[dk][:, ts * 512:(ts + 1) * 512],
                                    start=(dk == 0), stop=(dk == 7))
                            # rot: out = praw*cosT + shuffle(praw)*sinT
                            praw = tmpb_pool.tile([128, 512], BFD, tag="praw")
                            nc.scalar.copy(out=praw, in_=ps)
                            shuf = tmpb_pool.tile([128, 512], BFD, tag="shuf")
                            nc.vector.stream_shuffle(shuf, praw, swap_mask)
                            nc.vector.tensor_tensor(
                                praw, praw, cosT_sb[:, ts * 512:(ts + 1) * 512], MULT)
                            nc.vector.tensor_tensor(
                                shuf, shuf, sinT_sb[:, ts * 512:(ts + 1) * 512], MULT)
                            nc.vector.tensor_tensor(
                                qt[:, ts * 512:(ts + 1) * 512], praw, shuf, ADD)

                    return qk_tiles

                def emit_attn(c2, qk_tiles):
                    # ---- attention per head ----
                    qv = qk_tiles["q"].rearrange("p (i a) -> p i a", a=4)
                    kv = qk_tiles["k"].rearrange("p (c j a) -> p c j a", c=4, j=128, a=4)
                    for hh in range(2):
                        h = 2 * c2 + hh
                        plo, phi = hh * 64, (hh + 1) * 64
                        expts_h = []
                        for Jc in range(4):
                            sim_ps = ps_pool.tile([128, 512], FP32, tag="ps", name="sim")
                            for a in range(BLK):
                                nc.tensor.matmul(
                                    sim_ps,
                                    kv[plo:phi, Jc, :, a],
                                    qv[plo:phi, :, a],
                                    start=(a == 0), stop=(a == 3))
                            et = exp_pool.tile([128, 512], BFD, tag="expt")
                            nc.scalar.activation(
                                out=et, in_=sim_ps,
                                func=mybir.ActivationFunctionType.Exp,
                                scale=SCALE)
                            expts_h.append(et)
                        # PV matmuls do not need the normalizer; start them now
                        pv_pss = []
                        for cp in range(2):
                            pv_ps = ps_pool.tile([128, 512], FP32, tag="ps",
                                                 name=f"pv{cp}")
                            for Jc in range(4):
                                lhsT = vb_sb[Jc].rearrange(
                                    "p (h a d) -> p h a d", h=16, a=4, d=64)[
                                        :, h, 2 * cp:2 * cp + 2, :]
                                nc.tensor.matmul(
                                    pv_ps, lhsT, expts_h[Jc],
                                    start=(Jc == 0), stop=(Jc == 3))
                            pv_pss.append(pv_ps)
                        sums_ps = ps_pool.tile([128, 512], FP32, tag="ps", name="sums")
                        for Jc in range(4):
                            nc.tensor.matmul(
                                sums_ps, ones_sb, expts_h[Jc],
                                start=(Jc == 0), stop=(Jc == 3))
                        nc.scalar.activation(
                            out=sums_ps, in_=sums_ps,
                            func=mybir.ActivationFunctionType.Ln)
                        rsums = tmpa_pool.tile([128, 512], BFD, tag="rsums")
                        nc.scalar.activation(
                            out=rsums, in_=sums_ps,
                            func=mybir.ActivationFunctionType.Exp, scale=-1.0)

                        for cp in range(2):  # d'-chunk (a-pair 2cp, 2cp+1)
                            pv_ps = pv_pss[cp]
                            # normalize, inv-translate, inv-rotate, interleave out
                            asb = tmpa_pool.tile([128, 512], BFD, tag="asb")
                            nc.vector.tensor_tensor(asb, pv_ps, rsums, MULT)
                            a2 = tmpa_pool.tile([128, 512], BFD, tag="a2")
                            nc.vector.tensor_tensor(a2, asb, transB_sb[cp], SUB)
                            shf = tmpa_pool.tile([128, 512], BFD, tag="shf")
                            nc.vector.stream_shuffle(shf, a2, swap_mask)
                            u1 = tmpa_pool.tile([128, 512], BFD, tag="u1")
                            u2 = tmpa_pool.tile([128, 512], BFD, tag="u2")
                            nc.vector.tensor_tensor(u1, a2, cosE_sb[cp], MULT)
                            nc.gpsimd.tensor_tensor(u2, shf, sinE_sb[cp], MULT)
                            aov = ao_sb[c2].rearrange("p (a i) -> p a i", a=4)
                            for ap2 in range(2):
                                nc.vector.tensor_tensor(
                                    aov[plo:phi, 2 * cp + ap2, :],
                                    u1[ap2 * 64:(ap2 + 1) * 64, :],
                                    u2[ap2 * 64:(ap2 + 1) * 64, :],
                                    ADD)

                prev = None
                for c2 in range(8):
                    qk_t = emit_proj(c2)
                    if prev is not None:
                        emit_attn(prev[0], prev[1])
                    prev = (c2, qk_t)
                emit_attn(prev[0], prev[1])

                # ================= output projection ========================
                out_v = out_d[b].rearrange("(i a) e -> i a e", a=4)
                groups = [(a, cI, esl) for a in range(4) for cI in range(4)
                          for esl in range(2)]
                NWAVE = 5
                wave = []
                for gi in range(NWAVE):
                    a, cI, esl = groups[gi]
                    ps = ps_pool.tile([128, 512], FP32, tag="ps", name=f"fw{gi}")
                    for jc in range(7):
                        nc.tensor.matmul(
                            ps,
                            ao_sb[jc][:, a * 512 + cI * 128:
                                      a * 512 + (cI + 1) * 128],
                            wout_sb[jc][:, esl * 512:(esl + 1) * 512],
                            start=(jc == 0), stop=False)
                    wave.append(ps)
                for gi in range(NWAVE):
                    a, cI, esl = groups[gi]
                    nc.tensor.matmul(
                        wave[gi],
                        ao_sb[7][:, a * 512 + cI * 128:a * 512 + (cI + 1) * 128],
                        wout_sb[7][:, esl * 512:(esl + 1) * 512],
                        start=False, stop=True)
                    oev = oev_pool.tile([128, 512], BFD, tag="oev")
                    nc.vector.tensor_tensor(
                        oev, wave[gi], bout_sb[:, esl * 512:(esl + 1) * 512], ADD)
                    nc.sync.dma_start(
                        out=out_v[cI * 128:(cI + 1) * 128, a,
                                  esl * 512:(esl + 1) * 512],
                        in_=oev)
                for gi in range(NWAVE, len(groups)):
                    a, cI, esl = groups[gi]
                    ps = ps_pool.tile([128, 512], FP32, tag="ps")
                    for jc in range(8):
                        nc.tensor.matmul(
                            ps,
                            ao_sb[jc][:, a * 512 + cI * 128:
                                      a * 512 + (cI + 1) * 128],
                            wout_sb[jc][:, esl * 512:(esl + 1) * 512],
                            start=(jc == 0), stop=(jc == 7))
                    oev = oev_pool.tile([128, 512], BFD, tag="oev")
                    nc.vector.tensor_tensor(
                        oev, ps, bout_sb[:, esl * 512:(esl + 1) * 512], ADD)
                    nc.sync.dma_start(
                        out=out_v[cI * 128:(cI + 1) * 128, a,
                                  esl * 512:(esl + 1) * 512],
                        in_=oev)
    _split_multi_waits(nc)
    return nc


def _host_prep(x, angles, trans, W_qkv, W_out, b_out, trans_coeff):
    """Build all per-core input arrays (layout/dtype staging + cos/sin coeffs)."""
    c = float(np.asarray(trans_coeff).reshape(-1)[0])
    cos = np.cos(angles).astype(np.float32)   # [B, N, 16]
    sin = np.sin(angles).astype(np.float32)

    xT = np.ascontiguousarray(x.transpose(0, 2, 1)).astype(BF16)       # [B, DIM, N]
    wqkvT = np.ascontiguousarray(np.asarray(W_qkv).T).astype(BF16)     # [DIM, 3HDH]
    woutT = np.ascontiguousarray(np.asarray(W_out).T).astype(BF16)     # [DIM, DIM]
    boutB = np.ascontiguousarray(
        np.broadcast_to(np.asarray(b_out)[None, :], (128, DIM))).astype(np.float32)

    dh = np.arange(DH)
    pair_idx = np.clip((dh - D_FLAT) // 2, 0, NPAIR - 1)               # [64]
    is_rot = dh >= D_FLAT
    is_odd = ((dh - D_FLAT) % 2 == 1) & is_rot

    # ---- cosT/sinT [B, 128, N]: rows = (half, dh); fwd rotation, [j,t] layout
    base_cos = np.where(is_rot[None, None, :], cos[:, :, pair_idx], 1.0)  # [B,N,64]
    sgn = np.where(is_rot, np.where(is_odd, 1.0, -1.0), 0.0)
    base_sin = sin[:, :, pair_idx] * sgn[None, None, :]
    cosT = np.tile(base_cos.transpose(0, 2, 1), (1, 2, 1)).astype(BF16)   # [B,128,N]
    sinT = np.tile(base_sin.transpose(0, 2, 1), (1, 2, 1)).astype(BF16)

    # ---- cosN/sinN [B, BLK, NB, 128] for V: cols (h=8, i=16)
    J = np.arange(NB)
    cstN = np.empty((B, BLK, NB, 512), np.float32)
    for a in range(BLK):
        t_idx = 4 * J + a
        cstN[:, a, :, 0:128] = np.tile(cos[:, t_idx, :], (1, 1, 8))
        cstN[:, a, :, 128:256] = np.tile(sin[:, t_idx, :], (1, 1, 8))
        cstN[:, a, :, 256:512] = np.tile(c * np.asarray(trans)[:, t_idx, :], (1, 1, 8))
    cstN = cstN.astype(BF16)

    # ---- inverse coeffs [B, 2, 128, NB]: rows = (a2, dh); t = 4I + 2*c2 + a2
    cosE = np.empty((B, 2, 128, NB), np.float32)
    sinE = np.empty((B, 2, 128, NB), np.float32)
    transB = np.zeros((B, 2, 128, NB), np.float32)  # cast to bf16 below
    I = np.arange(NB)
    sgnE = np.where(is_rot, np.where(is_odd, -1.0, 1.0), 0.0)
    for c2 in range(2):
        for a2 in range(2):
            t_idx = 4 * I + 2 * c2 + a2
            cc = cos[:, t_idx, :][:, :, pair_idx].transpose(0, 2, 1)   # [B,64,NB]
            ss = sin[:, t_idx, :][:, :, pair_idx].transpose(0, 2, 1)
            cosE[:, c2, a2 * 64:(a2 + 1) * 64, :] = np.where(
                is_rot[None, :, None], cc, 1.0)
            sinE[:, c2, a2 * 64:(a2 + 1) * 64, :] = ss * sgnE[None, :, None]
            tb = c * np.asarray(trans)[:, t_idx, :].transpose(0, 2, 1)  # [B,32,NB]
            transB[:, c2, a2 * 64 + D_FLAT:(a2 + 1) * 64, :] = tb
    cosE = cosE.astype(BF16)
    sinE = sinE.astype(BF16)

    return dict(xT=xT, wqkvT=wqkvT, woutT=woutT, boutB=boutB,
                cosT=cosT, sinT=sinT, cstN=cstN,
                cosE=cosE, sinE=sinE, transB=transB.astype(BF16))


def kernel(x, angles, trans, W_qkv, W_out, b_out, trans_coeff, _profile=False):
    x = np.asarray(x)
    angles = np.asarray(angles)
    trans = np.asarray(trans)
    arrs = _host_prep(x, angles, trans, W_qkv, W_out, b_out, trans_coeff)
    if "nc" not in _CACHE:
        _CACHE["nc"] = _build_nc()
    nc = _CACHE["nc"]

    in_maps = []
    for core in range(NCORES):
        bsl = slice(core * B2, (core + 1) * B2)
        m = dict(
            xT=np.ascontiguousarray(arrs["xT"][bsl]),
            wqkvT=arrs["wqkvT"], woutT=arrs["woutT"], boutB=arrs["boutB"],
            cosT=np.ascontiguousarray(arrs["cosT"][bsl]),
            sinT=np.ascontiguousarray(arrs["sinT"][bsl]),
            cstN=np.ascontiguousarray(arrs["cstN"][bsl]),
            cosE=np.ascontiguousarray(arrs["cosE"][bsl]),
            sinE=np.ascontiguousarray(arrs["sinE"][bsl]),
            transB=np.ascontiguousarray(arrs["transB"][bsl]),
        )
        in_maps.append(m)

    res = run_bass_kernel_spmd(nc, in_maps, core_ids=list(range(NCORES)),
                               trace=_profile)
    out = np.concatenate([r["out"] for r in res.results], axis=0).astype(np.float32)
    if _profile:
        _CACHE["last_exec_time_ns"] = res.exec_time_ns
        _CACHE["last_trace"] = res.instructions_and_trace
    return out

